# revision 1
# baseline (speedup 1.0000x reference)
import numpy as np
import jax
import jax.numpy as jnp
from functools import partial

# KalmanNet gain network, data-parallel over batch on 8 NeuronCores.
# B=32768 sharded 8 ways (4096/core); per-d parameters replicated.
B, D, M, N = 32768, 16, 2, 1
NCORES = 8
EPS = 1e-6

_PARAM_KEYS = [
    'fc1_w', 'fc1_b', 'fc2_w', 'fc2_b', 'fc3_w', 'fc3_b', 'fc4_w', 'fc4_b',
    'fc5a_w', 'fc5a_b', 'fc5b_w', 'fc5b_b', 'fc6_w', 'fc6_b', 'fc7_w', 'fc7_b',
    'gru1_wih', 'gru1_whh', 'gru1_bih', 'gru1_bhh',
    'gru2_wih', 'gru2_whh', 'gru2_bih', 'gru2_bhh',
    'gru3_wih', 'gru3_whh', 'gru3_bih', 'gru3_bhh',
]
_BATCH_KEYS = ['del_y_til', 'del_y', 'del_x_til', 'del_x_hat', 'Q', 'Sigma', 'S']


def _lin(x, w, b):
    return jnp.einsum('bdi,doi->bdo', x, w) + b


def _fc(x, w, b):
    return jax.nn.relu(_lin(x, w, b))


def _l2norm(x):
    nrm = jnp.sqrt(jnp.sum(x * x, axis=-1, keepdims=True))
    return x / jnp.maximum(nrm, EPS)


def _gru_step(x, h, wih, whh, bih, bhh):
    gi = jnp.einsum('bdi,dgi->bdg', x, wih) + bih
    gh = jnp.einsum('bdh,dgh->bdg', h, whh) + bhh
    ir, iz, i_n = jnp.split(gi, 3, axis=-1)
    hr, hz, h_n = jnp.split(gh, 3, axis=-1)
    r = jax.nn.sigmoid(ir + hr)
    z = jax.nn.sigmoid(iz + hz)
    cand = jnp.tanh(i_n + r * h_n)
    return (1.0 - z) * cand + z * h


def _forward(batch, params):
    (del_y_til, del_y, del_x_til, del_x_hat, Q, Sigma, S) = batch
    p = dict(zip(_PARAM_KEYS, params))
    in1 = _l2norm(_fc(del_x_hat, p['fc1_w'], p['fc1_b']))
    Qn = _gru_step(in1, Q, p['gru1_wih'], p['gru1_whh'],
                   p['gru1_bih'], p['gru1_bhh'])
    in2 = _l2norm(jnp.concatenate(
        [Qn, _fc(del_x_til, p['fc2_w'], p['fc2_b'])], axis=-1))
    Sigman = _gru_step(in2, Sigma, p['gru2_wih'], p['gru2_whh'],
                       p['gru2_bih'], p['gru2_bhh'])
    in3 = _l2norm(jnp.concatenate([
        _fc(Sigman, p['fc3_w'], p['fc3_b']),
        _fc(jnp.concatenate([del_y_til, del_y], axis=-1),
            p['fc4_w'], p['fc4_b'])], axis=-1))
    Sn = _gru_step(in3, S, p['gru3_wih'], p['gru3_whh'],
                   p['gru3_bih'], p['gru3_bhh'])
    cat_ss = jnp.concatenate([Sigman, Sn], axis=-1)
    K = _lin(jax.nn.relu(_lin(cat_ss, p['fc5a_w'], p['fc5a_b'])),
             p['fc5b_w'], p['fc5b_b'])
    Sigma_next = _fc(jnp.concatenate(
        [Sigman, _fc(jnp.concatenate([Sn, K], axis=-1), p['fc6_w'], p['fc6_b'])],
        axis=-1), p['fc7_w'], p['fc7_b'])
    return jnp.concatenate([K, Qn, Sigma_next, Sn], axis=-1)


_pmapped = None
_param_cache = {}


def _get_pmapped():
    global _pmapped
    if _pmapped is None:
        devs = jax.devices()[:NCORES]
        _pmapped = jax.pmap(_forward, axis_name='cores', devices=devs)
    return _pmapped


def _device_params(inputs):
    # Replicate the small per-d parameter stack onto every core once;
    # reuse device buffers across calls when the host arrays are unchanged.
    key = tuple(id(inputs[k]) for k in _PARAM_KEYS)
    if _param_cache.get('key') != key:
        devs = jax.devices()[:NCORES]
        _param_cache['val'] = [
            jax.device_put_replicated(np.asarray(inputs[k]), devs)
            for k in _PARAM_KEYS]
        _param_cache['key'] = key
    return _param_cache['val']


def kernel(**inputs):
    # Pure data parallel: shard the batch axis across the 8 cores.
    batch = [np.asarray(inputs[k]).reshape(NCORES, B // NCORES,
                                           *inputs[k].shape[1:])
             for k in _BATCH_KEYS]
    out = _get_pmapped()(batch, _device_params(inputs))
    return np.asarray(out).reshape(B, D, 11)



# revision 40
# speedup vs baseline: 1.3644x; 1.3644x over previous
"""KalmanNet gain-network Trainium2 Bass kernel.

Data-parallel over batch on 8 NeuronCores (4096 batch columns/core).
On-chip layout is feature-major: SBUF rows = (source d, feature), columns =
batch.  All 16 d-slices are packed into block-diagonal weight matrices built
on the host, so every PE matmul streams 512 batch columns at a time.
Activations are bf16 (PSUM accumulates f32); l2-norm scales are applied after
the gi matmuls (l2norm(x) @ W == (x @ W) * s), with the per-sample scale
broadcast across gate rows by DMA.

Host side packs inputs f32 [B,D,f] -> bf16 feature-major [rows, B] and
unpacks the bf16 [176, B] output back to f32 [B, D, 11].
"""

import numpy as np
import ml_dtypes

BF16 = ml_dtypes.bfloat16
B, D = 32768, 16
NCORES = 8
BC = B // NCORES          # batch columns per core
NCH = 512                 # columns per chunk (one PSUM bank)

# d-groups for the 40-wide layers (3 d per 128-row window, padded)
DG = [(0, 3), (3, 6), (6, 9), (9, 12), (12, 15), (15, 16)]

IN_ROWS = 244
OUT_ROWS = 176            # d*11 + [K(2), Qn(4), Sigma_next(4), Sn(1)]

_PARAM_KEYS = [
    'fc1_w', 'fc1_b', 'fc2_w', 'fc2_b', 'fc3_w', 'fc3_b', 'fc4_w', 'fc4_b',
    'fc5a_w', 'fc5a_b', 'fc5b_w', 'fc5b_b', 'fc6_w', 'fc6_b', 'fc7_w', 'fc7_b',
    'gru1_wih', 'gru1_whh', 'gru1_bih', 'gru1_bhh',
    'gru2_wih', 'gru2_whh', 'gru2_bih', 'gru2_bhh',
    'gru3_wih', 'gru3_whh', 'gru3_bih', 'gru3_bhh',
]


# --------------------------------------------------------------------------
# matmul stream plan + packed weight blob
# --------------------------------------------------------------------------

def _plan_streams(wd):
    streams = []
    blobs = []
    col = [0]

    def add(src, K, psum, p0, p1, lhsT, start=True, stop=True):
        M = p1 - p0
        assert lhsT.shape == (K, M), (src, psum, lhsT.shape, (K, M))
        streams.append(dict(src=src, K=K, M=M, psum=psum, p0=p0, p1=p1,
                            start=start, stop=stop, col=col[0]))
        blobs.append(lhsT.astype(np.float32))
        col[0] += M

    f32 = np.float32

    # ---- fc1/fc2/fc4: 2 -> 40 (+bias), relu later; blocked by d-group ----
    def fc124(name_w, name_b, xrow0, psname):
        w = wd[name_w]; b = wd[name_b]
        for g, (d0, d1) in enumerate(DG):
            lhsT = np.zeros((97, 128), f32)
            for d in range(d0, d1):
                for o in range(40):
                    m = (d - d0) * 40 + o
                    lhsT[xrow0 + d*2 + 0, m] = w[d][o, 0]
                    lhsT[xrow0 + d*2 + 1, m] = w[d][o, 1]
                    lhsT[96, m] = b[d][o]
            add('XT', 97, f'{psname}{g}', 0, 128, lhsT)

    fc124('fc1_w', 'fc1_b', 0, 'U1b')
    fc124('fc2_w', 'fc2_b', 32, 'U2b')
    fc124('fc4_w', 'fc4_b', 64, 'U4b')

    # ---- sum-of-squares reductions into SS psum --------------------------
    # SS rows: ss1 [0:16], ss2 [32:48], ss3 [64:80], fc3 pre-act [96:112]
    def ssq_windows(srcfmt, width, ssp0, extra_first=None):
        first = True
        if extra_first is not None:
            src, K, lhsT = extra_first
            add(src, K, 'SS', ssp0, ssp0 + 16, lhsT, start=True, stop=False)
            first = False
        for g, (d0, d1) in enumerate(DG):
            lhsT = np.zeros((128, 16), f32)
            for d in range(d0, d1):
                lhsT[(d - d0)*width:(d - d0)*width + width, d] = 1.0
            last = g == len(DG) - 1
            add(srcfmt.format(g=g), 128, 'SS', ssp0, ssp0 + 16, lhsT,
                start=first, stop=last)
            first = False

    ssq_windows('SQ1b{g}', 40, 0)

    # ---- GRU 1/2 (h=4): gi gate-major + gh -------------------------------
    def gru12(i, wih_k, whh_k, bih_k, bhh_k, h_src, in_cols, parts):
        wih = wd[wih_k]; whh = wd[whh_k]; bihv = wd[bih_k]; bhhv = wd[bhh_k]
        nparts = len(parts)
        for pi, (src, K, rowmap) in enumerate(parts):
            lrz = np.zeros((K, 128), f32)
            ln = np.zeros((K, 64), f32)
            for d in range(D):
                for c in range(in_cols):
                    r = rowmap(d, c)
                    if r is None:
                        continue
                    for j in range(4):
                        lrz[r, d*4 + j] = wih[d][j, c]
                        lrz[r, 64 + d*4 + j] = wih[d][4 + j, c]
                        ln[r, d*4 + j] = wih[d][8 + j, c]
            add(src, K, f'P{i}rz', 0, 128, lrz,
                start=(pi == 0), stop=(pi == nparts - 1))
            add(src, K, f'P{i}nv', 0, 64, ln,
                start=(pi == 0), stop=(pi == nparts - 1))
        K = 65
        lrz = np.zeros((K, 128), f32)
        ln = np.zeros((K, 64), f32)
        for d in range(D):
            for j in range(4):
                for k in range(4):
                    lrz[d*4 + k, d*4 + j] = whh[d][j, k]
                    lrz[d*4 + k, 64 + d*4 + j] = whh[d][4 + j, k]
                    ln[d*4 + k, d*4 + j] = whh[d][8 + j, k]
                lrz[64, d*4 + j] = bhhv[d][j] + bihv[d][j]
                lrz[64, 64 + d*4 + j] = bhhv[d][4 + j] + bihv[d][4 + j]
                ln[64, d*4 + j] = bhhv[d][8 + j]
        add(h_src, K, f'V{i}rz', 0, 128, lrz)
        add(h_src, K, f'P{i}nv', 64, 128, ln)

    def dgrp_map(g, width, coff):
        d0, d1 = DG[g]
        def f(d, c):
            if c >= coff and d0 <= d < d1:
                return (d - d0)*width + (c - coff)
            return None
        return f

    gru12(1, 'gru1_wih', 'gru1_whh', 'gru1_bih', 'gru1_bhh', 'H1', 40,
          [(f'R1b{g}', 128, dgrp_map(g, 40, 0)) for g in range(6)])

    lq = np.zeros((64, 16), f32)
    for d in range(D):
        lq[d*4:d*4+4, d] = 1.0
    ssq_windows('SQ2b{g}', 40, 32, extra_first=('SQQn', 64, lq))

    qn_part = ('QnT', 64, lambda d, c: d*4 + c if c < 4 else None)
    gru12(2, 'gru2_wih', 'gru2_whh', 'gru2_bih', 'gru2_bhh', 'H2', 44,
          [qn_part] + [(f'R2b{g}', 128, dgrp_map(g, 40, 4)) for g in range(6)])

    # ---- fc3: Sigman -> 1 (+bias); contracts over C5T rows [0:65] -------
    # C5 layout: [0:64] Sigman (d*4+c), [64] ones, [65:81] Sn (d)
    w3 = wd['fc3_w']; b3 = wd['fc3_b']
    lhsT = np.zeros((65, 16), f32)
    for d in range(D):
        for c in range(4):
            lhsT[d*4 + c, d] = w3[d][0, c]
        lhsT[64, d] = b3[d][0]
    add('C5F3', 65, 'F3PS', 0, 16, lhsT)

    lf = np.eye(16, dtype=f32)
    ssq_windows('SQ4b{g}', 40, 64, extra_first=('SQF3', 16, lf))

    # ---- gru3 (h=1): gates padded to 32-row slots (r@0, z@32, n@64) ------
    wih = wd['gru3_wih']; whh = wd['gru3_whh']
    bihv = wd['gru3_bih']; bhhv = wd['gru3_bhh']
    parts3 = [('F3T', 16, lambda d, c: d if c == 0 else None)] + \
             [(f'R4b{g}', 128, dgrp_map(g, 40, 1)) for g in range(6)]
    for pi, (src, K, rowmap) in enumerate(parts3):
        l3 = np.zeros((K, 80), f32)
        for d in range(D):
            for c in range(41):
                r = rowmap(d, c)
                if r is None:
                    continue
                l3[r, d] = wih[d][0, c]
                l3[r, 32 + d] = wih[d][1, c]
                l3[r, 64 + d] = wih[d][2, c]
        add(src, K, 'P3A', 0, 80, l3, start=(pi == 0),
            stop=(pi == len(parts3)-1))
    l3 = np.zeros((17, 80), f32)
    for d in range(D):
        l3[d, d] = whh[d][0, 0]
        l3[d, 32 + d] = whh[d][1, 0]
        l3[d, 64 + d] = whh[d][2, 0]
        l3[16, d] = bhhv[d][0] + bihv[d][0]
        l3[16, 32 + d] = bhhv[d][1] + bihv[d][1]
        l3[16, 64 + d] = bhhv[d][2]
    add('H3', 17, 'P3B', 0, 80, l3)

    # ---- fc5a ------------------------------------------------------------
    w5a = wd['fc5a_w']; b5a = wd['fc5a_b']
    for g in range(13):
        lhsT = np.zeros((81, 128), f32)
        for m in range(128):
            h = 128*g + m
            if h >= 1600:
                break
            d, o = divmod(h, 100)
            for c in range(4):
                lhsT[d*4 + c, m] = w5a[d][o, c]
            lhsT[65 + d, m] = w5a[d][o, 4]
            lhsT[64, m] = b5a[d][o]
        add('C5T', 81, f'H5ps{g}', 0, 128, lhsT)

    # ---- fc5b ------------------------------------------------------------
    w5b = wd['fc5b_w']; b5b = wd['fc5b_b']
    for g in range(13):
        lhsT = np.zeros((128, 32), f32)
        for m in range(128):
            h = 128*g + m
            if h >= 1600:
                break
            d, o = divmod(h, 100)
            lhsT[m, d*2 + 0] = w5b[d][0, o]
            lhsT[m, d*2 + 1] = w5b[d][1, o]
        add(f'H5b{g}', 128, 'KPS', 0, 32, lhsT, start=(g == 0), stop=False)
    lhsT = np.zeros((1, 32), f32)
    for d in range(D):
        lhsT[0, d*2 + 0] = b5b[d][0]
        lhsT[0, d*2 + 1] = b5b[d][1]
    add('ONEROW', 1, 'KPS', 0, 32, lhsT, start=False, stop=True)

    # ---- fc6: [Sn, K] -> 4, relu; F67 rows [0:64] ------------------------
    w6 = wd['fc6_w']; b6 = wd['fc6_b']
    lhsT = np.zeros((17, 64), f32)
    for d in range(D):
        for o in range(4):
            lhsT[d, d*4 + o] = w6[d][o, 0]
            lhsT[16, d*4 + o] = b6[d][o]
    add('SnT', 17, 'F67', 0, 64, lhsT, start=True, stop=False)
    lhsT = np.zeros((32, 64), f32)
    for d in range(D):
        for o in range(4):
            lhsT[d*2 + 0, d*4 + o] = w6[d][o, 1]
            lhsT[d*2 + 1, d*4 + o] = w6[d][o, 2]
    add('KT', 32, 'F67', 0, 64, lhsT, start=False, stop=True)

    # ---- fc7: [Sigman, F6] -> 4, relu; F67 rows [64:128] -----------------
    w7 = wd['fc7_w']; b7 = wd['fc7_b']
    lhsT = np.zeros((81, 64), f32)
    for d in range(D):
        for o in range(4):
            for c in range(4):
                lhsT[d*4 + c, d*4 + o] = w7[d][o, c]
            lhsT[64, d*4 + o] = b7[d][o]
    add('C5T', 81, 'F67', 64, 128, lhsT, start=True, stop=False)
    lhsT = np.zeros((64, 64), f32)
    for d in range(D):
        for o in range(4):
            for e in range(4):
                lhsT[d*4 + e, d*4 + o] = w7[d][o, 4 + e]
    add('F6T', 64, 'F67', 64, 128, lhsT, start=False, stop=True)

    wcol = col[0]
    wei = np.zeros((128, wcol), np.float32)
    for s, blob in zip(streams, blobs):
        wei[:s['K'], s['col']:s['col'] + s['M']] = blob
    return streams, wei


# --------------------------------------------------------------------------
# device program
# --------------------------------------------------------------------------

def _build_nc(streams, wcol, bc):
    import concourse.bass as bass
    import concourse.tile as tile
    from concourse import bacc
    from concourse import mybir

    nchunks = bc // NCH
    f32 = mybir.dt.float32
    bf = mybir.dt.bfloat16
    AF = mybir.ActivationFunctionType
    OP = mybir.AluOpType

    nc = bacc.Bacc("TRN2", target_bir_lowering=False)
    IN = nc.dram_tensor("IN", [IN_ROWS, bc], bf, kind="ExternalInput")
    WEI = nc.dram_tensor("WEI", [128, wcol], bf, kind="ExternalInput")
    BIH1 = nc.dram_tensor("BIH1", [64, 1], f32, kind="ExternalInput")
    BIH2 = nc.dram_tensor("BIH2", [64, 1], f32, kind="ExternalInput")
    BIH3 = nc.dram_tensor("BIH3", [16, 1], f32, kind="ExternalInput")
    OUT = nc.dram_tensor("OUT", [OUT_ROWS, bc], bf, kind="ExternalOutput")

    by_ps = {}
    for s in streams:
        by_ps.setdefault(s['psum'], []).append(s)

    def sbuf_ap(t, row0, nrows, off_cols, ap_dims):
        base = t[:]
        return bass.AP(tensor=base.tensor,
                       offset=base.offset + row0 * NCH + off_cols,
                       ap=ap_dims)

    with tile.TileContext(nc) as tc:
        with tc.tile_pool(name="const", bufs=1) as cpool, \
             tc.tile_pool(name="inp", bufs=2) as inpool, \
             tc.tile_pool(name="acts", bufs=2) as apool, \
             tc.tile_pool(name="gates", bufs=1) as gpool, \
             tc.tile_pool(name="ps_u", bufs=2, space="PSUM") as ppu, \
             tc.tile_pool(name="ps_g", bufs=1, space="PSUM") as ppg, \
             tc.tile_pool(name="ps_ss", bufs=1, space="PSUM") as pps:

            WS = cpool.tile([128, wcol], bf)
            nc.sync.dma_start(out=WS, in_=WEI[:])
            B1 = cpool.tile([64, 1], f32)
            nc.sync.dma_start(out=B1, in_=BIH1[:])
            B2 = cpool.tile([64, 1], f32)
            nc.sync.dma_start(out=B2, in_=BIH2[:])
            B3 = cpool.tile([80, 1], f32)
            nc.sync.dma_start(out=B3[64:80], in_=BIH3[:])
            EPS = cpool.tile([16, 1], f32)
            nc.vector.memset(EPS, 1e-12)

            for ch in range(nchunks):
                c0 = ch * NCH

                def mm(s, ps, rhs):
                    nc.tensor.matmul(
                        ps[s['p0']:s['p1']],
                        WS[0:s['K'], s['col']:s['col'] + s['M']],
                        rhs,
                        start=s['start'], stop=s['stop'])

                XT = inpool.tile([97, NCH], bf, tag='XT')
                nc.sync.dma_start(out=XT[:], in_=IN[0:97, c0:c0+NCH])
                H1 = inpool.tile([65, NCH], bf, tag='H1')
                nc.sync.dma_start(out=H1[:], in_=IN[97:162, c0:c0+NCH])
                H2 = inpool.tile([65, NCH], bf, tag='H2')
                nc.sync.dma_start(out=H2[:], in_=IN[162:227, c0:c0+NCH])
                H3 = inpool.tile([17, NCH], bf, tag='H3')
                nc.sync.dma_start(out=H3[:], in_=IN[227:244, c0:c0+NCH])
                ONEROW = inpool.tile([1, NCH], bf, tag='ONEROW')
                nc.sync.dma_start(out=ONEROW[:], in_=IN[96:97, c0:c0+NCH])

                SS = pps.tile([80, NCH], f32, tag='SS')
                SRT = gpool.tile([80, NCH], f32, tag='SRT')

                RT = {}
                SQT = {}
                for ups, rn, sn in [('U1b', 'R1', 'SQ1'), ('U2b', 'R2', 'SQ2'),
                                    ('U4b', 'R4', 'SQ4')]:
                    RT[rn] = apool.tile([128, 6, NCH], bf, tag=rn, name=rn)
                    for g in range(6):
                        ps = ppu.tile([128, NCH], f32, tag='u124')
                        mm(by_ps[f'{ups}{g}'][0], ps, XT[:])
                        nc.scalar.activation(out=RT[rn][:, g, :], in_=ps[:],
                                             func=AF.Relu)
                    SQT[sn] = apool.tile([128, 6, NCH], bf, tag=sn, name=sn)
                    nc.vector.tensor_tensor(out=SQT[sn][:], in0=RT[rn][:],
                                            in1=RT[rn][:], op=OP.mult)

                def norm_scale(ssp0, srow):
                    # s = 1/sqrt(ss + 1e-12) in one ACT op (table-accurate
                    # to ~4e-5 rel; the custom-DVE reciprocal is avoided --
                    # its ISA-lowered APs evade Tile dependency tracking)
                    nc.scalar.activation(out=SRT[srow:srow+16],
                                         in_=SS[ssp0:ssp0+16],
                                         func=AF.Abs_reciprocal_sqrt,
                                         bias=EPS[:])

                def bcast_s(srow, dstt, dst0, nrep, ngrp=1):
                    # dst rows [dst0:dst0+16*nrep*ngrp) <- SRT row srow+d,
                    # value repeated nrep times per d (and the whole block
                    # ngrp times)
                    dims = [[0, ngrp]] if ngrp > 1 else []
                    dims += [[NCH, 16]]
                    if nrep > 1:
                        dims += [[0, nrep]]
                    dims += [[1, NCH]]
                    src = sbuf_ap(SRT, srow, 16, 0, dims)
                    dstap = sbuf_ap(dstt, dst0, 16*nrep*ngrp, 0,
                                    [[NCH, 16*nrep*ngrp], [1, NCH]])
                    nc.sync.dma_start(out=dstap, in_=src)

                # ---- ss1 -> s1
                for s in by_ps['SS'][0:6]:
                    mm(s, SS, SQT['SQ1'][:, int(s['src'][-1]), :])
                norm_scale(0, 0)
                S1B = gpool.tile([128, NCH], f32, tag='S1B')
                bcast_s(0, S1B, 0, 4)
                bcast_s(0, S1B, 64, 4)
                S1N = gpool.tile([64, NCH], f32, tag='S1N')
                bcast_s(0, S1N, 0, 4)

                # ---- gru1 matmuls
                P1rz = ppg.tile([128, NCH], f32, tag='Prz')
                P1nv = ppg.tile([128, NCH], f32, tag='Pnv')
                V1rz = ppg.tile([128, NCH], f32, tag='Vrz')
                for s in by_ps['P1rz']:
                    mm(s, P1rz, RT['R1'][:, int(s['src'][-1]), :])
                for s in by_ps['P1nv']:
                    rhs = (H1[:] if s['src'] == 'H1'
                           else RT['R1'][:, int(s['src'][-1]), :])
                    mm(s, P1nv, rhs)
                mm(by_ps['V1rz'][0], V1rz, H1[:])

                def gru_gates(i, Prz, Pnv, Vrz, SB, SBN, bih, hsrc, hn_out,
                              hr):
                    Grz = gpool.tile([2*hr, NCH], bf, tag=f'Grz{i}')
                    nc.vector.tensor_tensor(out=Grz[:], in0=Prz, in1=SB,
                                            op=OP.mult)
                    Gn = gpool.tile([hr, NCH], bf, tag=f'Gn{i}')
                    nc.vector.tensor_tensor(out=Gn[:], in0=Pnv, in1=SBN,
                                            op=OP.mult)
                    Arz = gpool.tile([2*hr, NCH], bf, tag=f'Arz{i}')
                    nc.vector.tensor_tensor(out=Arz[:], in0=Grz[:], in1=Vrz,
                                            op=OP.add)
                    rz = gpool.tile([2*hr, NCH], bf, tag=f'rz{i}')
                    nc.scalar.activation(out=rz[:], in_=Arz[:],
                                         func=AF.Sigmoid)
                    t2 = gpool.tile([hr, NCH], bf, tag=f't2{i}')
                    nc.vector.tensor_tensor(out=t2[:], in0=rz[0:hr],
                                            in1=Vn_ap, op=OP.mult)
                    An = gpool.tile([hr, NCH], bf, tag=f'An{i}')
                    nc.vector.scalar_tensor_tensor(
                        out=An[:], in0=Gn[:], scalar=bih, in1=t2[:],
                        op0=OP.add, op1=OP.add)
                    cand = gpool.tile([hr, NCH], bf, tag=f'cand{i}')
                    nc.scalar.activation(out=cand[:], in_=An[:], func=AF.Tanh)
                    # t4 lives at base hr so the z-multiply has equal SBUF
                    # input bases (HW verifier constraint)
                    t4 = gpool.tile([2*hr, NCH], bf, tag=f't4{i}')
                    nc.vector.scalar_tensor_tensor(
                        out=t4[hr:2*hr], in0=cand[:], scalar=-1.0, in1=hsrc,
                        op0=OP.mult, op1=OP.add)
                    t5 = gpool.tile([hr, NCH], bf, tag=f't5{i}')
                    nc.vector.tensor_tensor(out=t5[:], in0=rz[hr:2*hr],
                                            in1=t4[hr:2*hr], op=OP.mult)
                    nc.vector.tensor_tensor(out=hn_out, in0=t5[:],
                                            in1=cand[:], op=OP.add)

                QnT = gpool.tile([64, NCH], bf, tag='QnT')
                Vn_ap = P1nv[64:128]
                gru_gates(1, P1rz[:], P1nv[0:64], V1rz[:], S1B[:], S1N[:],
                          B1[:], H1[0:64], QnT[:], 64)

                # ---- ss2 -> s2
                SQQn = gpool.tile([64, NCH], bf, tag='SQQn')
                nc.vector.tensor_tensor(out=SQQn[:], in0=QnT[:], in1=QnT[:],
                                        op=OP.mult)
                ss2 = by_ps['SS'][6:13]
                mm(ss2[0], SS, SQQn[:])
                for s in ss2[1:]:
                    mm(s, SS, SQT['SQ2'][:, int(s['src'][-1]), :])
                norm_scale(32, 32)
                S2B = gpool.tile([128, NCH], f32, tag='S2B')
                bcast_s(32, S2B, 0, 4)
                bcast_s(32, S2B, 64, 4)
                S2N = gpool.tile([64, NCH], f32, tag='S2N')
                bcast_s(32, S2N, 0, 4)

                # ---- gru2
                P2rz = ppg.tile([128, NCH], f32, tag='Prz')
                P2nv = ppg.tile([128, NCH], f32, tag='Pnv')
                V2rz = ppg.tile([128, NCH], f32, tag='Vrz')
                for s in by_ps['P2rz']:
                    rhs = (QnT[:] if s['src'] == 'QnT'
                           else RT['R2'][:, int(s['src'][-1]), :])
                    mm(s, P2rz, rhs)
                for s in by_ps['P2nv']:
                    rhs = (QnT[:] if s['src'] == 'QnT'
                           else H2[:] if s['src'] == 'H2'
                           else RT['R2'][:, int(s['src'][-1]), :])
                    mm(s, P2nv, rhs)
                mm(by_ps['V2rz'][0], V2rz, H2[:])

                C5T = gpool.tile([81, NCH], bf, tag='C5T')
                Vn_ap = P2nv[64:128]
                gru_gates(2, P2rz[:], P2nv[0:64], V2rz[:], S2B[:], S2N[:],
                          B2[:], H2[0:64], C5T[0:64], 64)
                nc.sync.dma_start(out=C5T[64:65], in_=IN[96:97, c0:c0+NCH])

                # ---- fc3 + relu + square
                F3ps = ppu.tile([16, NCH], f32, tag='f67', bufs=1)
                mm(by_ps['F3PS'][0], F3ps, C5T[0:65])
                F3T = gpool.tile([16, NCH], bf, tag='F3T')
                nc.scalar.activation(out=F3T[:], in_=F3ps[:], func=AF.Relu)
                SQF3 = gpool.tile([16, NCH], bf, tag='SQF3')
                nc.vector.tensor_tensor(out=SQF3[:], in0=F3T[:], in1=F3T[:],
                                        op=OP.mult)

                # ---- ss3 -> s3
                ss3 = by_ps['SS'][13:20]
                mm(ss3[0], SS, SQF3[:])
                for s in ss3[1:]:
                    mm(s, SS, SQT['SQ4'][:, int(s['src'][-1]), :])
                norm_scale(64, 64)
                S3B = gpool.tile([80, NCH], f32, tag='S3B')
                for g3 in range(5):
                    nc.sync.dma_start(out=S3B[g3*16:g3*16+16],
                                      in_=SRT[64:80])

                # ---- gru3 (gates at 32-aligned slots: r@0, z@32, n@64)
                P3a = ppg.tile([80, NCH], f32, tag='Prz')
                P3b = ppg.tile([80, NCH], f32, tag='Pnv')
                for s in by_ps['P3A']:
                    rhs = (F3T[:] if s['src'] == 'F3T'
                           else RT['R4'][:, int(s['src'][-1]), :])
                    mm(s, P3a, rhs)
                mm(by_ps['P3B'][0], P3b, H3[:])
                G3 = gpool.tile([80, NCH], bf, tag='G3')
                nc.vector.tensor_tensor(out=G3[:], in0=P3a[:], in1=S3B[:],
                                        op=OP.mult)
                A3 = gpool.tile([64, NCH], bf, tag='A3')
                nc.vector.tensor_tensor(out=A3[:], in0=G3[0:64],
                                        in1=P3b[0:64], op=OP.add)
                rz3 = gpool.tile([64, NCH], bf, tag='rz3')
                nc.scalar.activation(out=rz3[:], in_=A3[:], func=AF.Sigmoid)
                t23 = gpool.tile([80, NCH], bf, tag='t23')
                nc.vector.tensor_tensor(out=t23[64:80], in0=rz3[0:16],
                                        in1=P3b[64:80], op=OP.mult)
                An3 = gpool.tile([16, NCH], bf, tag='An3')
                nc.vector.scalar_tensor_tensor(
                    out=An3[:], in0=G3[64:80], scalar=B3[64:80],
                    in1=t23[64:80], op0=OP.add, op1=OP.add)
                c3 = gpool.tile([16, NCH], bf, tag='c3')
                nc.scalar.activation(out=c3[:], in_=An3[:], func=AF.Tanh)
                t43 = gpool.tile([48, NCH], bf, tag='t43')
                nc.vector.scalar_tensor_tensor(
                    out=t43[32:48], in0=c3[:], scalar=-1.0, in1=H3[0:16],
                    op0=OP.mult, op1=OP.add)
                t53 = gpool.tile([16, NCH], bf, tag='t53')
                nc.vector.tensor_tensor(out=t53[:], in0=rz3[32:48],
                                        in1=t43[32:48], op=OP.mult)
                SnT = gpool.tile([17, NCH], bf, tag='SnT')
                nc.vector.tensor_tensor(out=SnT[0:16], in0=t53[:], in1=c3[:],
                                        op=OP.add)
                nc.sync.dma_start(out=SnT[16:17], in_=IN[96:97, c0:c0+NCH])

                # ---- finish C5 (rows [64:80] = Sn)
                nc.sync.dma_start(out=C5T[65:81], in_=SnT[0:16])

                # ---- fc5a -> H5
                H5T = apool.tile([128, 13, NCH], bf, tag='H5T')
                for g in range(13):
                    ps = ppu.tile([128, NCH], f32, tag='u124')
                    mm(by_ps[f'H5ps{g}'][0], ps, C5T[:])
                    nc.scalar.activation(out=H5T[:, g, :], in_=ps[:],
                                         func=AF.Relu)

                # ---- fc5b -> K
                KPS = ppu.tile([32, NCH], f32, tag='kps', bufs=1)
                for s in by_ps['KPS']:
                    rhs = (ONEROW[:] if s['src'] == 'ONEROW'
                           else H5T[:, int(s['src'][3:]), :])
                    mm(s, KPS, rhs)
                KT = gpool.tile([32, NCH], bf, tag='KT')
                nc.scalar.activation(out=KT[:], in_=KPS[:], func=AF.Copy)

                # ---- fc6 / fc7
                F67 = ppu.tile([128, NCH], f32, tag='f67', bufs=1)
                s6 = by_ps['F67']
                mm(s6[0], F67, SnT[:])
                mm(s6[1], F67, KT[:])
                F6T = gpool.tile([64, NCH], bf, tag='F6T')
                nc.scalar.activation(out=F6T[:], in_=F67[0:64], func=AF.Relu)
                mm(s6[2], F67, C5T[:])
                mm(s6[3], F67, F6T[:])
                F7T = gpool.tile([64, NCH], bf, tag='F7T')
                nc.scalar.activation(out=F7T[:], in_=F67[64:128],
                                     func=AF.Relu)

                # ---- output stores
                def store(srcap, r0, ncomp):
                    dst = bass.AP(
                        tensor=OUT[:].tensor, offset=r0 * bc + c0,
                        ap=[[11 * bc, 16], [bc, ncomp], [1, NCH]])
                    nc.sync.dma_start(out=dst, in_=srcap)

                store(KT[:], 0, 2)
                store(QnT[:], 2, 4)
                store(F7T[:], 6, 4)
                store(SnT[0:16], 10, 1)

    nc.compile()
    return nc


# --------------------------------------------------------------------------
# host packing / unpacking
# --------------------------------------------------------------------------

def _pack_inputs(inputs, nb=B):
    IN_all = np.empty((IN_ROWS, nb), BF16)

    def fm(name, f):
        return np.asarray(inputs[name]).transpose(1, 2, 0).reshape(D * f, nb)

    IN_all[0:32] = fm('del_x_hat', 2)
    IN_all[32:64] = fm('del_x_til', 2)
    yt = np.asarray(inputs['del_y_til']).transpose(1, 2, 0).reshape(D, nb)
    y = np.asarray(inputs['del_y']).transpose(1, 2, 0).reshape(D, nb)
    IN_all[64:96] = np.stack([yt, y], axis=1).reshape(32, nb)
    IN_all[96] = 1.0
    IN_all[97:161] = fm('Q', 4)
    IN_all[161] = 1.0
    IN_all[162:226] = fm('Sigma', 4)
    IN_all[226] = 1.0
    IN_all[227:243] = fm('S', 1)
    IN_all[243] = 1.0
    return IN_all


def _unpack_output(out_all, nb=B):
    return np.ascontiguousarray(
        out_all.reshape(D, 11, nb).transpose(2, 0, 1)).astype(np.float32)


def _consts_from(wd):
    bih1 = np.stack([wd['gru1_bih'][d][8:12] for d in range(D)]
                    ).reshape(64, 1).astype(np.float32)
    bih2 = np.stack([wd['gru2_bih'][d][8:12] for d in range(D)]
                    ).reshape(64, 1).astype(np.float32)
    bih3 = np.asarray([wd['gru3_bih'][d][2] for d in range(D)]
                      ).reshape(16, 1).astype(np.float32)
    return bih1, bih2, bih3


# --------------------------------------------------------------------------
# execution via PJRT (axon) with cached jit
# --------------------------------------------------------------------------

_STATE = {}


def _get_program(inputs, bc=BC):
    key = tuple(id(np.asarray(inputs[k])) for k in _PARAM_KEYS[:4]) + (bc,)
    st = _STATE.get('prog')
    if st is not None and st['key'] == key:
        return st
    wd = {k: np.asarray(inputs[k]) for k in _PARAM_KEYS}
    streams, wei = _plan_streams(wd)
    nc = _build_nc(streams, wei.shape[1], bc)
    bih1, bih2, bih3 = _consts_from(wd)
    st = dict(key=key, bc=bc, streams=streams, wei=wei.astype(BF16),
              bih1=bih1, bih2=bih2, bih3=bih3, nc=nc, jit=None)
    _STATE['prog'] = st
    return st


def _make_jit(st):
    import jax
    import numpy as _np
    from jax.sharding import Mesh, PartitionSpec
    from jax.experimental.shard_map import shard_map
    import jax.numpy as jnp
    from concourse.bass2jax import _bass_exec_p, install_neuronx_cc_hook
    from concourse import mybir

    install_neuronx_cc_hook()
    nc = st['nc']
    pid_name = (nc.partition_id_tensor.name
                if nc.partition_id_tensor is not None else None)
    in_names, out_names, out_avals, zero_shapes = [], [], [], []
    for alloc in nc.m.functions[0].allocations:
        if not isinstance(alloc, mybir.MemoryLocationSet):
            continue
        name = alloc.memorylocations[0].name
        if alloc.kind == "ExternalInput":
            if name == pid_name:
                continue
            in_names.append(name)
        elif alloc.kind == "ExternalOutput":
            out_names.append(name)
            dt = mybir.dt.np(alloc.dtype)
            out_avals.append(
                jax.core.ShapedArray(tuple(alloc.tensor_shape), dt))
            zero_shapes.append((tuple(alloc.tensor_shape), dt))
    n_params = len(in_names)
    n_outs = len(out_names)
    all_names = in_names + out_names
    if pid_name is not None:
        all_names = all_names + [pid_name]

    def _body(*args):
        from concourse.bass2jax import partition_id_tensor
        operands = list(args)
        if pid_name is not None:
            operands.append(partition_id_tensor())
        outs = _bass_exec_p.bind(
            *operands,
            out_avals=tuple(out_avals),
            in_names=tuple(all_names),
            out_names=tuple(out_names),
            lowering_input_output_aliases=(),
            sim_require_finite=True,
            sim_require_nnan=True,
            nc=nc,
        )
        return tuple(outs)

    devices = jax.devices()[:NCORES]
    mesh = Mesh(_np.asarray(devices), ("core",))
    donate = tuple(range(n_params, n_params + n_outs))
    sharded = jax.jit(
        shard_map(_body, mesh=mesh,
                  in_specs=(PartitionSpec("core"),) * (n_params + n_outs),
                  out_specs=(PartitionSpec("core"),) * n_outs,
                  check_rep=False),
        donate_argnums=donate,
        keep_unused=True,
    )

    # device-side creation of the donated zero output buffers (no host
    # transfer per call)
    from jax.sharding import NamedSharding
    shardings = [NamedSharding(mesh, PartitionSpec("core"))] * len(zero_shapes)
    zero_fn = jax.jit(
        lambda: tuple(jnp.zeros((NCORES * s[0], *s[1:]), d)
                      for s, d in zero_shapes),
        out_shardings=tuple(shardings))
    st['jit'] = (sharded, zero_fn, in_names, out_names)
    return st['jit']


def kernel(**inputs):
    st = _get_program(inputs)
    if st['jit'] is None:
        _make_jit(st)
    sharded, zero_fn, in_names, out_names = st['jit']

    IN_all = _pack_inputs(inputs)
    per_core = {
        'IN': np.ascontiguousarray(
            IN_all.reshape(IN_ROWS, NCORES, BC).transpose(1, 0, 2)
        ).reshape(NCORES * IN_ROWS, BC),
        'WEI': np.ascontiguousarray(
            np.broadcast_to(st['wei'], (NCORES,) + st['wei'].shape)
        ).reshape(NCORES * 128, -1),
        'BIH1': np.ascontiguousarray(
            np.broadcast_to(st['bih1'], (NCORES, 64, 1))).reshape(-1, 1),
        'BIH2': np.ascontiguousarray(
            np.broadcast_to(st['bih2'], (NCORES, 64, 1))).reshape(-1, 1),
        'BIH3': np.ascontiguousarray(
            np.broadcast_to(st['bih3'], (NCORES, 16, 1))).reshape(-1, 1),
    }
    args = [per_core[n] for n in in_names]
    outs = sharded(*args, *zero_fn())
    out = np.asarray(outs[out_names.index('OUT')])  # [8*176, BC]
    out_all = np.concatenate(
        [out[i*OUT_ROWS:(i+1)*OUT_ROWS] for i in range(NCORES)], axis=1)
    return _unpack_output(out_all)
